# revision 9
# baseline (speedup 1.0000x reference)
"""CapsuleLayer (dynamic routing) Trainium2 kernel.

Full inputs -> batch-sharded over 8 NeuronCores -> full output.

Math (per sample b):
    ihat[i,c,o] = sum_d x[i,d] * W[i,c,d,o]
    bias = 0
    for r in 0..2:
        coup = softmax(bias, axis=c)
        s[c,o] = sum_i coup[i,c] * ihat[i,c,o]
        v = squash(s)
        if r < 2: bias[i,c] += sum_o ihat[i,c,o] * v[c,o]
    return v

Device layout (per core, 32 local samples, batch-tiles of 8):
    SBUF partition dim p = (b, i_sub): p = b*16 + i_sub   (b in 0..7 of tile,
    i_sub = i % 16), free dim (ig, c, o) with ig = i // 16 (72 groups).
    ihat tile: [128, 72*10*16]

    einsum: per (bt, ig) one matmul, lhsT = host-prepared block-diagonal
    x tile [ (i_sub,d)=128, (b,i_sub')=128 ], rhs = W chunk [128, 160].

    routing weighted sum: per ig matmul with lhsT = coupZ [128, (c',b')=80]
    (coup placed on the b'=b diagonal via a mask multiply), PSUM-accumulated
    over ig. The [80,160] result holds s[b,c,:] in its (c'==c) diagonal
    blocks; it is masked on evacuation, squashed with per-partition scalars,
    and collapsed to [8,160] with a selector matmul (engine partition ranges
    must start 32-aligned, so no sub-32 partition slicing anywhere).
"""

import sys

if "/opt/trn_rl_repo" not in sys.path:
    sys.path.insert(0, "/opt/trn_rl_repo")

import numpy as np

B, I, D, C, O = 256, 1152, 8, 10, 16
NCORES = 8
BL = B // NCORES            # 32 local samples per core
NBT, BT = 4, 8              # batch tiles
ISUB = 16                   # i's per group
IG = I // ISUB              # 72 groups
CO = C * O                  # 160
NR = 3
EPS = 1e-7
XZ_CHUNK = 18               # ig's per xz DMA chunk
F32 = np.float32

_compiled = {}


def _build_program():
    import concourse.bacc as bacc
    import concourse.tile as tile
    import concourse.mybir as mybir
    import concourse.bass as bass

    f32 = mybir.dt.float32
    nc = bacc.Bacc("TRN2", target_bir_lowering=False, debug=False,
                   num_devices=NCORES)

    xz_t = nc.dram_tensor("xz", [NBT * IG, 128, 128], f32, kind="ExternalInput")
    xt_t = nc.dram_tensor("xt", [128, IG, BL], f32, kind="ExternalInput")
    w_t = nc.dram_tensor("w", [128, IG * CO], f32, kind="ExternalInput")
    cmask_t = nc.dram_tensor("cmask", [C * BT, CO], f32, kind="ExternalInput")
    maskz_t = nc.dram_tensor("maskz", [128, C * BT], f32, kind="ExternalInput")
    sel_t = nc.dram_tensor("sel", [C * BT, BT], f32, kind="ExternalInput")
    out_t = nc.dram_tensor("out", [BL, CO], f32, kind="ExternalOutput")
    vscr_t = nc.dram_tensor("vscr", [BL, CO], f32)   # internal scratch
    xz_ap, xt_ap, w_ap = xz_t.ap(), xt_t.ap(), w_t.ap()
    out_ap, vscr_ap = out_t.ap(), vscr_t.ap()

    AF = mybir.ActivationFunctionType
    ALU = mybir.AluOpType
    AX = mybir.AxisListType

    with tile.TileContext(nc) as tc:
        from contextlib import ExitStack

        with ExitStack() as ctx:
            singles = ctx.enter_context(tc.tile_pool(name="singles", bufs=1))
            xzp = ctx.enter_context(tc.tile_pool(name="xzp", bufs=3))
            psum = ctx.enter_context(tc.tile_pool(name="psum", bufs=4, space="PSUM"))
            psm = ctx.enter_context(tc.tile_pool(name="psm", bufs=1, space="PSUM"))
            tch = ctx.enter_context(tc.tile_pool(name="tch", bufs=2))
            sm = ctx.enter_context(tc.tile_pool(name="sm", bufs=2))

            w_sb = singles.tile([128, IG * CO], f32)
            nc.sync.dma_start(out=w_sb, in_=w_ap)
            xt_sb = singles.tile([128, IG * BL], f32)
            nc.sync.dma_start(out=xt_sb,
                              in_=xt_ap.rearrange("p g b -> p (g b)"))
            cmask = singles.tile([C * BT, CO], f32)
            nc.sync.dma_start(out=cmask, in_=cmask_t.ap())
            maskz = singles.tile([128, C * BT], f32)
            nc.sync.dma_start(out=maskz, in_=maskz_t.ap())
            sel_sb = singles.tile([C * BT, BT], f32)
            nc.sync.dma_start(out=sel_sb, in_=sel_t.ap())

            ihat = singles.tile([128, IG * CO], f32)
            zsc = singles.tile([128, IG * C * BT], f32)     # coupZ
            bias = singles.tile([128, IG * C], f32)
            tmp720 = singles.tile([128, IG * C], f32)
            coup = singles.tile([128, IG * C], f32)
            zsum = singles.tile([128, IG], f32)
            vrep = singles.tile([128, CO], f32)

            # ---- r0 weighted sum: s0 = 0.1 * sum_{i,d} x*W  (all 32 b) ----
            ps0 = psm.tile([BL, CO], f32)
            for kc in range(IG):
                nc.tensor.matmul(ps0, xt_sb[:, kc * BL:(kc + 1) * BL],
                                 w_sb[:, kc * CO:(kc + 1) * CO],
                                 start=(kc == 0), stop=(kc == IG - 1))
            s_all = singles.tile([BL, CO], f32)
            nc.scalar.mul(s_all, ps0, 1.0 / C)

            # ---- squash32: reference squash on a [32, (c,o)] tile --------
            def nr_rsqrt(pool, a, p, w):
                """exact-ish rsqrt(a) via Sqrt table seed + 2 Newton steps"""
                sq = pool.tile([p, w], f32)
                nc.scalar.activation(sq, a, AF.Sqrt)
                rs = pool.tile([p, w], f32)
                nc.vector.reciprocal(rs, sq)
                t1 = pool.tile([p, w], f32)
                t2 = pool.tile([p, w], f32)
                for _ in range(2):
                    nc.vector.tensor_mul(t1, a, rs)
                    nc.vector.tensor_mul(t1, t1, rs)
                    nc.vector.tensor_scalar(t2, t1, -0.5, 1.5,
                                            op0=ALU.mult, op1=ALU.add)
                    nc.vector.tensor_mul(rs, rs, t2)
                return rs

            def squash_factor(pool, n2, p, w):
                """f = n2 / ((1+n2) * sqrt(n2+eps)), elementwise [p, w]"""
                a = pool.tile([p, w], f32)
                nc.vector.tensor_scalar_add(a, n2, EPS)
                rs = nr_rsqrt(pool, a, p, w)
                dn = pool.tile([p, w], f32)
                nc.vector.tensor_scalar_add(dn, n2, 1.0)
                di = pool.tile([p, w], f32)
                nc.vector.reciprocal(di, dn)
                f = pool.tile([p, w], f32)
                nc.vector.tensor_mul(f, n2, rs)
                nc.vector.tensor_mul(f, f, di)
                return f

            # r0 squash on [32, CO]
            sq32 = singles.tile([BL, CO], f32)
            nc.vector.tensor_mul(sq32, s_all, s_all)
            n2_32 = singles.tile([BL, C], f32)
            nc.vector.tensor_reduce(
                n2_32, sq32.rearrange("p (c o) -> p c o", c=C),
                axis=AX.X, op=ALU.add)
            f32t = squash_factor(singles, n2_32, BL, C)
            v0 = singles.tile([BL, CO], f32)
            fb = bass.AP(tensor=f32t.tensor, offset=f32t.offset,
                         ap=[f32t.ap[0], f32t.ap[1], [0, O]])
            nc.vector.tensor_tensor(v0, s_all, fb, op=ALU.mult)
            nc.sync.dma_start(out=vscr_ap, in_=v0)

            for bt in range(NBT):
                # ================= einsum: ihat for this batch tile =========
                for ch in range(IG // XZ_CHUNK):
                    xz_sb = xzp.tile([128, XZ_CHUNK * 128], f32)
                    base = bt * IG + ch * XZ_CHUNK
                    nc.sync.dma_start(
                        out=xz_sb.rearrange("p (t m) -> p t m", t=XZ_CHUNK),
                        in_=xz_ap[base:base + XZ_CHUNK].rearrange(
                            "t p m -> p t m"))
                    for t in range(XZ_CHUNK):
                        ig = ch * XZ_CHUNK + t
                        pih = psum.tile([128, CO], f32)
                        nc.tensor.matmul(pih, xz_sb[:, t * 128:(t + 1) * 128],
                                         w_sb[:, ig * CO:(ig + 1) * CO],
                                         start=True, stop=True)
                        dst = ihat[:, ig * CO:(ig + 1) * CO]
                        if ig % 2 == 0:
                            nc.vector.tensor_copy(dst, pih)
                        else:
                            nc.scalar.copy(dst, pih)

                vsrc = None   # None -> use vscr dram rows for this bt (r0)
                for r in range(NR - 1):
                    # ---- vrep[p=(b,i_sub), co] = v[b, co] ------------------
                    if vsrc is None:
                        vi = bass.AP(tensor=vscr_ap.tensor,
                                     offset=bt * BT * CO,
                                     ap=[[CO, BT], [0, ISUB], [1, CO]])
                    else:
                        vi = bass.AP(tensor=vsrc.tensor, offset=vsrc.offset,
                                     ap=[vsrc.ap[0], [0, ISUB], [1, CO]])
                    nc.gpsimd.dma_start(out=vrep, in_=vi)
                    # ---- bias (+)= sum_o ihat * vrep -----------------------
                    for ch in range(4):
                        g0 = ch * (IG // 4)
                        gn = IG // 4
                        tc_t = tch.tile([128, gn * CO], f32)
                        vb = bass.AP(tensor=vrep.tensor, offset=vrep.offset,
                                     ap=[vrep.ap[0], [0, gn], [1, CO]])
                        nc.vector.tensor_tensor(
                            tc_t, ihat[:, g0 * CO:(g0 + gn) * CO], vb,
                            op=ALU.mult)
                        red_dst = (bias if r == 0 else tmp720)[
                            :, g0 * C:(g0 + gn) * C]
                        nc.vector.tensor_reduce(
                            red_dst,
                            tc_t.rearrange("p (gc o) -> p gc o", o=O),
                            axis=AX.X, op=ALU.add)
                    if r > 0:
                        nc.vector.tensor_add(bias, bias, tmp720)

                    # ---- coup = softmax(bias) over c -----------------------
                    nc.scalar.activation(coup, bias, AF.Exp)
                    nc.vector.tensor_reduce(
                        zsum, coup.rearrange("p (g c) -> p g c", c=C),
                        axis=AX.X, op=ALU.add)
                    rz = sm.tile([128, IG], f32)
                    nc.vector.reciprocal(rz, zsum)
                    rzb = bass.AP(tensor=rz.tensor, offset=rz.offset,
                                  ap=[rz.ap[0], rz.ap[1], [0, C]])
                    nc.vector.tensor_tensor(coup, coup, rzb, op=ALU.mult)

                    # ---- zsc[(b,i),(g,c,b')] = coup[(b,i),(g,c)]*d(b,b') ---
                    zr = zsc.rearrange("p (g c b) -> p g c b", c=C, b=BT)
                    cr = coup.rearrange("p (g c) -> p g c", c=C)
                    cb = bass.AP(tensor=cr.tensor, offset=cr.offset,
                                 ap=[cr.ap[0], cr.ap[1], cr.ap[2], [0, BT]])
                    mr = maskz.rearrange("p (c b) -> p c b", b=BT)
                    mb = bass.AP(tensor=mr.tensor, offset=mr.offset,
                                 ap=[mr.ap[0], [0, IG], mr.ap[1], mr.ap[2]])
                    nc.vector.tensor_tensor(zr, cb, mb, op=ALU.mult)

                    # ---- s = sum_i coup*ihat via PE ------------------------
                    pss = psm.tile([C * BT, CO], f32)
                    for ig in range(IG):
                        nc.tensor.matmul(
                            pss, zsc[:, ig * C * BT:(ig + 1) * C * BT],
                            ihat[:, ig * CO:(ig + 1) * CO],
                            start=(ig == 0), stop=(ig == IG - 1))
                    # masked evacuation: sst[(c',b),(c,o)] = pss * d(c,c')
                    sst = sm.tile([C * BT, CO], f32)
                    nc.vector.tensor_tensor(sst, pss, cmask, op=ALU.mult)
                    # n2 per partition (c',b):  sum over free of sst^2
                    sjunk = sm.tile([C * BT, CO], f32)
                    n2_80 = sm.tile([C * BT, 1], f32)
                    nc.vector.scalar_tensor_tensor(
                        sjunk, sst, 1.0, sst, op0=ALU.mult, op1=ALU.mult,
                        accum_out=n2_80)
                    f80 = squash_factor(sm, n2_80, C * BT, 1)
                    v80 = sm.tile([C * BT, CO], f32)
                    nc.vector.tensor_scalar_mul(v80, sst, f80)
                    # collapse (c',b) -> b with selector matmul
                    v8ps = psm.tile([BT, CO], f32)
                    nc.tensor.matmul(v8ps, sel_sb, v80, start=True, stop=True)
                    v_sb = sm.tile([BT, CO], f32)
                    nc.vector.tensor_copy(v_sb, v8ps)
                    vsrc = v_sb

                nc.sync.dma_start(out=out_ap[bt * BT:(bt + 1) * BT, :],
                                  in_=vsrc)

    nc.compile()
    return nc


def _prep_inputs(x, W):
    """Host-side layout transforms (not part of measured HW time)."""
    x = np.ascontiguousarray(x, dtype=F32)
    W = np.ascontiguousarray(W, dtype=F32)
    # W -> [(i_sub, d), (ig, c, o)]
    wr = np.ascontiguousarray(
        W.reshape(IG, ISUB, C, D, O).transpose(1, 3, 0, 2, 4)
    ).reshape(128, IG * CO)

    # x -> per core [core, bt, b, ig, i_sub, d]
    x8 = x.reshape(NCORES, NBT, BT, IG, ISUB, D)

    # block-diagonal lhsT tiles: xz[core, bt, ig, (i_sub,d), (b,i_sub')]
    xz = np.zeros((NCORES, NBT, IG, ISUB, D, 128), dtype=F32)
    isub = np.arange(ISUB)
    for b in range(BT):
        # advanced indexing pulls the i_sub axis to the front
        xz[:, :, :, isub, :, b * ISUB + isub] = \
            x8[:, :, b].transpose(3, 0, 1, 2, 4)
    xz = xz.reshape(NCORES, NBT * IG, 128, 128)

    # compact xT for r0: [core, (i_sub,d), ig, b]
    xt = np.ascontiguousarray(
        x8.reshape(NCORES, BL, IG, ISUB, D).transpose(0, 3, 4, 2, 1)
    ).reshape(NCORES, 128, IG, BL)

    # constants
    cmask = np.zeros((C * BT, CO), dtype=F32)       # [(c',b), (c,o)]
    for c in range(C):
        cmask[c * BT:(c + 1) * BT, c * O:(c + 1) * O] = 1.0
    # maskz[p=(b,i), (c,b')] = 1 iff b' == b
    maskz = np.zeros((128, C * BT), dtype=F32)      # [(b,i_sub), (c,b')]
    for b in range(BT):
        for c in range(C):
            maskz[b * ISUB:(b + 1) * ISUB, c * BT + b] = 1.0
    sel = np.zeros((C * BT, BT), dtype=F32)         # [(c',b), b']
    for c in range(C):
        for b in range(BT):
            sel[c * BT + b, b] = 1.0
    return xz, xt, wr, cmask, maskz, sel


def kernel(x: np.ndarray, W: np.ndarray) -> np.ndarray:
    from concourse import bass_utils

    if "nc" not in _compiled:
        _compiled["nc"] = _build_program()
    nc = _compiled["nc"]

    xz, xt, wr, cmask, maskz, sel = _prep_inputs(np.asarray(x), np.asarray(W))
    in_maps = [{"xz": xz[c], "xt": xt[c], "w": wr,
                "cmask": cmask, "maskz": maskz, "sel": sel}
               for c in range(NCORES)]
    res = bass_utils.run_bass_kernel_spmd(nc, in_maps, list(range(NCORES)))
    out = np.concatenate([res.results[c]["out"] for c in range(NCORES)], axis=0)
    return out.reshape(B, C, O)
